# revision 2
# baseline (speedup 1.0000x reference)
"""v3 Trainium2 kernel for nn_Group_10: bf16 matmuls + contiguous moving AP.

Same output mapping as baseline (PSUM comes out in pixel-shuffled order, store
is a strided DMA with 512B runs), but the x image is staged in SBUF as two
per-halfbank column-permuted slabs:

    slab_bk[ci, n, ct, 136*(col&7) + 4*hp + (col>>3)]
        = xpad[n, ct*128+ci, hp, 4*bk+col]          (col < 30)

so the matmul moving walk  f' = (w&7)*128 + 4h + (w>>3)  becomes the 2-dim AP
    [[136, 4], [1, 128]]   at offset 136*dx + 4*dy
with fully CONTIGUOUS 128-element (256B) inner runs: the inner merged index
4h + (w>>3) maps to slab address offset 4(h+dy) + (w>>3) which is stride-1,
and the w&7 quadrant (outer, 4 steps) is stride 136. The permuted map
(hp, col) -> 136*(col&7) + 4*hp + (col>>3) is injective and packs the
34x32 slab into exactly 1088 slots. Host precomputes the layout, so the
device-side x DMA is a plain contiguous copy. Weights/x in bf16; PSUM
accumulation and bias/store stay fp32.
"""

import numpy as np
import ml_dtypes
from contextlib import ExitStack

import concourse.bass as bass
import concourse.mybir as mybir
import concourse.tile as tile
from concourse import bacc
from concourse.bass_utils import run_bass_kernel_spmd

F32 = mybir.dt.float32
BF16 = mybir.dt.bfloat16
NPBF16 = ml_dtypes.bfloat16

N_CORES = 8
B = 4
CIN = 512
H = W_ = 32
COUT = 9728
NCHUNK = COUT // 64            # 152
CH_PER_CORE = NCHUNK // N_CORES  # 19
COUT_CORE = COUT // N_CORES    # 1216
NTILES = 10                    # 1216 padded to 1280 = 10 tiles of 128
HP = WP = 34                   # replicate-padded image
SLAB = HP * 32                 # 1088 elems per (n, ct, bk) slab
NCT = CIN // 128               # 4 cin tiles
XFREE = B * NCT * 2 * SLAB     # 34816 elems per partition

# within-tile PE output-partition permutation (same as baseline):
_m = np.arange(128)
_chunkbit, _rem = np.divmod(_m, 64)
_parity, _cchalf = np.divmod(_rem, 32)
COUT_IN_TILE = (_chunkbit * 64 + 2 * _cchalf + _parity).astype(np.int64)

_nc_cache = None


def _build_nc(rep=1, skip_mm=False, skip_out=False, order="nbk",
              xbufs=1, pbufs=4, oeng="scalar", wbufs=2, wpre=False,
              mergecb=False):
    xbufs, pbufs, wbufs = int(xbufs), int(pbufs), int(wbufs)
    nc = bacc.Bacc("TRN2", target_bir_lowering=False, debug=False,
                   num_devices=N_CORES)

    xs = nc.dram_tensor("xs", [128, XFREE], BF16, kind="ExternalInput")
    w = nc.dram_tensor("w", [NTILES, 128, NCT * 9 * 128], BF16,
                       kind="ExternalInput")
    bias = nc.dram_tensor("bias", [128, NTILES], F32, kind="ExternalInput")
    out = nc.dram_tensor("out", [B, CH_PER_CORE, 256, 256], F32,
                         kind="ExternalOutput")

    with ExitStack() as ctx:
        tc = ctx.enter_context(tile.TileContext(nc))
        xpool = ctx.enter_context(tc.tile_pool(name="xpool", bufs=xbufs))
        wpool = ctx.enter_context(tc.tile_pool(name="wpool", bufs=wbufs))
        opool = ctx.enter_context(tc.tile_pool(name="opool", bufs=3))
        bpool = ctx.enter_context(tc.tile_pool(name="bpool", bufs=1))
        ppool = ctx.enter_context(tc.tile_pool(name="ppool", bufs=pbufs,
                                               space="PSUM"))

        def body():
            x_sb = xpool.tile([128, XFREE], BF16)
            nc.sync.dma_start(x_sb, xs[:])
            xrow = x_sb.ap[0][0]
            xt = x_sb.tensor
            xoff0 = x_sb.offset

            bias_sb = bpool.tile([128, NTILES], F32)
            nc.sync.dma_start(bias_sb, bias[:])

            def rhs_ap(n, ct, tap, bk):
                dy, dx = divmod(tap, 3)
                base = ((n * NCT + ct) * 2 + bk) * SLAB
                return bass.AP(
                    xt,
                    xoff0 + base + 136 * dx + 4 * dy,
                    [[xrow, 128], [136, 4], [1, 128]],
                )

            out_eng = nc.scalar if oeng == "scalar" else nc.sync

            def store(t, n, psum):
                o_sb = opool.tile([128, 1024], F32)
                nc.vector.tensor_scalar_add(o_sb, psum, bias_sb[:, t:t + 1])
                if skip_out:
                    return
                orow = o_sb.ap[0][0]
                nchunks = 2 if t < NTILES - 1 else 1
                if mergecb and nchunks == 2:
                    src = bass.AP(o_sb.tensor, o_sb.offset,
                                  [[orow, 128], [128, 8], [1, 128]])
                    base = (n * CH_PER_CORE + 2 * t) * 65536
                    dst = bass.AP(out, base,
                                  [[65536, 2], [128, 2], [2048, 32],
                                   [256, 8], [1, 128]])
                    out_eng.dma_start(dst, src)
                    return
                for cb in range(nchunks):
                    src = bass.AP(o_sb.tensor, o_sb.offset + cb * 64 * orow,
                                  [[orow, 64], [128, 8], [1, 128]])
                    base = (n * CH_PER_CORE + 2 * t + cb) * 65536
                    dst = bass.AP(out, base,
                                  [[128, 2], [2048, 32], [256, 8], [1, 128]])
                    out_eng.dma_start(dst, src)

            w_tiles = {}

            def issue_w(tt):
                w_sb = wpool.tile([128, NCT * 9 * 128], BF16, name="w_sb")
                nc.sync.dma_start(w_sb, w[tt])
                w_tiles[tt] = w_sb

            L = int(wpre)
            for tt in range(min(L, NTILES)):
                issue_w(tt)

            for t in range(NTILES):
                if L:
                    if t + L < NTILES:
                        issue_w(t + L)
                    w_sb = w_tiles.pop(t)
                else:
                    w_sb = wpool.tile([128, NCT * 9 * 128], BF16,
                                      name="w_sb")
                    nc.sync.dma_start(w_sb, w[t])
                wrow = w_sb.ap[0][0]
                wt = w_sb.tensor
                woff = w_sb.offset

                def lhsT_ap(ct, tap):
                    return bass.AP(wt, woff + (ct * 9 + tap) * 128,
                                   [[wrow, 128], [1, 128]])

                def mm(psum, n, ct, tap, bk):
                    nc.tensor.matmul(
                        psum[:, 512 * bk:512 * (bk + 1)],
                        lhsT_ap(ct, tap),
                        rhs_ap(n, ct, tap, bk),
                        start=(tap == 0 and ct == 0),
                        stop=(tap == 8 and ct == 3),
                    )

                if order == "pair":
                    # weight-stationary across batch pairs: 4 MMs per
                    # weight tile; 2 PSUM tiles live per group, 2 draining
                    # (use pbufs=2: per-buf = 2 tiles = 4 banks)
                    for g in range(B // 2):
                        psums = [ppool.tile([128, 1024], F32, name="psp")
                                 for _ in range(2)]
                        if skip_mm:
                            for p in psums:
                                nc.vector.memset(p, 0.0)
                        else:
                            for tap in range(9):
                                for ct in range(NCT):
                                    for i in range(2):
                                        for bk in range(2):
                                            mm(psums[i], 2 * g + i,
                                               ct, tap, bk)
                        for i in range(2):
                            store(t, 2 * g + i, psums[i])
                elif order == "quad":
                    # weight-stationary across all 4 batches: 8 MMs per
                    # weight tile; all 4 PSUM tiles live (needs pbufs=4)
                    psums = [ppool.tile([128, 1024], F32, name=f"ps{i}")
                             for i in range(B)]
                    if skip_mm:
                        for p in psums:
                            nc.vector.memset(p, 0.0)
                    else:
                        for tap in range(9):
                            for ct in range(NCT):
                                for n in range(B):
                                    for bk in range(2):
                                        mm(psums[n], n, ct, tap, bk)
                    for n in range(B):
                        store(t, n, psums[n])
                else:
                    for n in range(B):
                        psum = ppool.tile([128, 1024], F32)
                        if skip_mm:
                            nc.vector.memset(psum, 0.0)
                        elif order == "bki":
                            # bk innermost: each weight tile loaded once
                            for tap in range(9):
                                for ct in range(NCT):
                                    for bk in range(2):
                                        mm(psum, n, ct, tap, bk)
                        else:
                            for bk in range(2):
                                for tap in range(9):
                                    for ct in range(NCT):
                                        mm(psum, n, ct, tap, bk)
                        store(t, n, psum)

        if rep == 1:
            body()
        else:
            with tc.For_i(0, rep):
                body()

    nc.compile()
    return nc


def _host_prep(x, W, b):
    xpad = np.pad(np.asarray(x, dtype=np.float32),
                  ((0, 0), (0, 0), (1, 1), (1, 1)), mode="edge")
    # [B, NCT, 128, 34, 34] -> [128, B, NCT, 34, 34]
    xr = np.ascontiguousarray(
        xpad.reshape(B, NCT, 128, HP, WP).transpose(2, 0, 1, 3, 4)
    ).astype(NPBF16)
    # slab[ci, n, ct, bk, a, 4*hp + r] = xpad[n, ..., hp, 4*bk + 8*r + a]
    xs = np.zeros((128, B, NCT, 2, 8, HP, 4), NPBF16)
    for bk in range(2):
        for a in range(8):
            for r in range(4):
                col = 8 * r + a
                wp = 4 * bk + col
                if col < 30 and wp < WP:
                    xs[:, :, :, bk, a, :, r] = xr[:, :, :, :, wp]
    xs = np.ascontiguousarray(xs.reshape(128, XFREE))

    W = np.asarray(W, dtype=np.float32)
    b = np.asarray(b, dtype=np.float32)

    in_maps = []
    for i in range(N_CORES):
        Ws = W[i * COUT_CORE:(i + 1) * COUT_CORE]          # [1216,512,3,3]
        Wp = np.zeros((NTILES * 128, CIN, 3, 3), np.float32)
        Wp[:COUT_CORE] = Ws
        gather = (np.arange(NTILES)[:, None] * 128 +
                  COUT_IN_TILE[None, :])                   # [10,128]
        Wg = Wp[gather]                                    # [10,128(m),512,3,3]
        Wg = Wg.reshape(NTILES, 128, NCT, 128, 9)          # [t,m,ct,p,tap]
        w_dev = np.ascontiguousarray(
            Wg.transpose(0, 3, 2, 4, 1)).astype(NPBF16)    # [t,p,ct,tap,m]
        w_dev = w_dev.reshape(NTILES, 128, NCT * 9 * 128)

        bp = np.zeros((NTILES * 128,), np.float32)
        bp[:COUT_CORE] = b[i * COUT_CORE:(i + 1) * COUT_CORE]
        bias_dev = np.ascontiguousarray(bp[gather].T)      # [128,10]

        in_maps.append({"xs": xs, "w": w_dev, "bias": bias_dev})
    return in_maps


def _run(in_maps, trace=False):
    global _nc_cache
    if _nc_cache is None:
        _nc_cache = _build_nc()
    return run_bass_kernel_spmd(_nc_cache, in_maps,
                                core_ids=list(range(N_CORES)), trace=trace)


def kernel(x, W, b):
    in_maps = _host_prep(x, W, b)
    res = _run(in_maps)
    outs = [res.results[i]["out"] for i in range(N_CORES)]  # [4,19,256,256]
    full = np.concatenate(outs, axis=1)                     # [4,152,256,256]
    return full


# revision 3
# speedup vs baseline: 1.0374x; 1.0374x over previous
"""v3 Trainium2 kernel for nn_Group_10: bf16 matmuls + contiguous moving AP.

Same output mapping as baseline (PSUM comes out in pixel-shuffled order, store
is a strided DMA with 512B runs), but the x image is staged in SBUF as two
per-halfbank column-permuted slabs:

    slab_bk[ci, n, ct, 136*(col&7) + 4*hp + (col>>3)]
        = xpad[n, ct*128+ci, hp, 4*bk+col]          (col < 30)

so the matmul moving walk  f' = (w&7)*128 + 4h + (w>>3)  becomes the 2-dim AP
    [[136, 4], [1, 128]]   at offset 136*dx + 4*dy
with fully CONTIGUOUS 128-element (256B) inner runs: the inner merged index
4h + (w>>3) maps to slab address offset 4(h+dy) + (w>>3) which is stride-1,
and the w&7 quadrant (outer, 4 steps) is stride 136. The permuted map
(hp, col) -> 136*(col&7) + 4*hp + (col>>3) is injective and packs the
34x32 slab into exactly 1088 slots. Host precomputes the layout, so the
device-side x DMA is a plain contiguous copy. Weights/x in bf16; PSUM
accumulation and bias/store stay fp32.
"""

import numpy as np
import ml_dtypes
from contextlib import ExitStack

import concourse.bass as bass
import concourse.mybir as mybir
import concourse.tile as tile
from concourse import bacc
from concourse.bass_utils import run_bass_kernel_spmd

F32 = mybir.dt.float32
BF16 = mybir.dt.bfloat16
NPBF16 = ml_dtypes.bfloat16

N_CORES = 8
B = 4
CIN = 512
H = W_ = 32
COUT = 9728
NCHUNK = COUT // 64            # 152
CH_PER_CORE = NCHUNK // N_CORES  # 19
COUT_CORE = COUT // N_CORES    # 1216
NTILES = 10                    # 1216 padded to 1280 = 10 tiles of 128
HP = WP = 34                   # replicate-padded image
SLAB = HP * 32                 # 1088 elems per (n, ct, bk) slab
NCT = CIN // 128               # 4 cin tiles
XFREE = B * NCT * 2 * SLAB     # 34816 elems per partition

# within-tile PE output-partition permutation (same as baseline):
_m = np.arange(128)
_chunkbit, _rem = np.divmod(_m, 64)
_parity, _cchalf = np.divmod(_rem, 32)
COUT_IN_TILE = (_chunkbit * 64 + 2 * _cchalf + _parity).astype(np.int64)

_nc_cache = None


def _build_nc(rep=1, skip_mm=False, skip_out=False, order="nbk",
              xbufs=1, pbufs=4, oeng="scalar", wbufs=3, wpre=2,
              mergecb=False, xsplit=True):
    xbufs, pbufs, wbufs = int(xbufs), int(pbufs), int(wbufs)
    nc = bacc.Bacc("TRN2", target_bir_lowering=False, debug=False,
                   num_devices=N_CORES)

    xs = nc.dram_tensor("xs", [128, XFREE], BF16, kind="ExternalInput")
    w = nc.dram_tensor("w", [NTILES, 128, NCT * 9 * 128], BF16,
                       kind="ExternalInput")
    bias = nc.dram_tensor("bias", [128, NTILES], F32, kind="ExternalInput")
    out = nc.dram_tensor("out", [B, CH_PER_CORE, 256, 256], F32,
                         kind="ExternalOutput")

    with ExitStack() as ctx:
        tc = ctx.enter_context(tile.TileContext(nc))
        xpool = ctx.enter_context(tc.tile_pool(name="xpool", bufs=xbufs))
        wpool = ctx.enter_context(tc.tile_pool(name="wpool", bufs=wbufs))
        opool = ctx.enter_context(tc.tile_pool(name="opool", bufs=3))
        bpool = ctx.enter_context(tc.tile_pool(name="bpool", bufs=1))
        ppool = ctx.enter_context(tc.tile_pool(name="ppool", bufs=pbufs,
                                               space="PSUM"))

        def body():
            x_sb = xpool.tile([128, XFREE], BF16)
            xrow = x_sb.ap[0][0]
            xt = x_sb.tensor
            xoff0 = x_sb.offset
            NXB = NCT * 2 * SLAB  # per-batch slab elems

            def dma_x(n):
                dst = bass.AP(xt, xoff0 + n * NXB, [[xrow, 128], [1, NXB]])
                src = bass.AP(xs, n * NXB, [[XFREE, 128], [1, NXB]])
                nc.sync.dma_start(dst, src)

            if not xsplit:
                nc.sync.dma_start(x_sb, xs[:])

            bias_sb = bpool.tile([128, NTILES], F32)
            nc.sync.dma_start(bias_sb, bias[:])

            def rhs_ap(n, ct, tap, bk):
                dy, dx = divmod(tap, 3)
                base = ((n * NCT + ct) * 2 + bk) * SLAB
                return bass.AP(
                    xt,
                    xoff0 + base + 136 * dx + 4 * dy,
                    [[xrow, 128], [136, 4], [1, 128]],
                )

            out_eng = nc.scalar if oeng == "scalar" else nc.sync

            def store(t, n, psum):
                o_sb = opool.tile([128, 1024], F32)
                nc.vector.tensor_scalar_add(o_sb, psum, bias_sb[:, t:t + 1])
                if skip_out:
                    return
                orow = o_sb.ap[0][0]
                nchunks = 2 if t < NTILES - 1 else 1
                if mergecb and nchunks == 2:
                    src = bass.AP(o_sb.tensor, o_sb.offset,
                                  [[orow, 128], [128, 8], [1, 128]])
                    base = (n * CH_PER_CORE + 2 * t) * 65536
                    dst = bass.AP(out, base,
                                  [[65536, 2], [128, 2], [2048, 32],
                                   [256, 8], [1, 128]])
                    out_eng.dma_start(dst, src)
                    return
                for cb in range(nchunks):
                    src = bass.AP(o_sb.tensor, o_sb.offset + cb * 64 * orow,
                                  [[orow, 64], [128, 8], [1, 128]])
                    base = (n * CH_PER_CORE + 2 * t + cb) * 65536
                    dst = bass.AP(out, base,
                                  [[128, 2], [2048, 32], [256, 8], [1, 128]])
                    out_eng.dma_start(dst, src)

            w_tiles = {}

            def issue_w(tt):
                w_sb = wpool.tile([128, NCT * 9 * 128], BF16, name="w_sb")
                nc.sync.dma_start(w_sb, w[tt])
                w_tiles[tt] = w_sb

            if xsplit:
                dma_x(0)
            L = int(wpre)
            for tt in range(min(L, NTILES)):
                issue_w(tt)
            if xsplit:
                for nn in range(1, B):
                    dma_x(nn)

            for t in range(NTILES):
                if L:
                    if t + L < NTILES:
                        issue_w(t + L)
                    w_sb = w_tiles.pop(t)
                else:
                    w_sb = wpool.tile([128, NCT * 9 * 128], BF16,
                                      name="w_sb")
                    nc.sync.dma_start(w_sb, w[t])
                wrow = w_sb.ap[0][0]
                wt = w_sb.tensor
                woff = w_sb.offset

                def lhsT_ap(ct, tap):
                    return bass.AP(wt, woff + (ct * 9 + tap) * 128,
                                   [[wrow, 128], [1, 128]])

                def mm(psum, n, ct, tap, bk):
                    nc.tensor.matmul(
                        psum[:, 512 * bk:512 * (bk + 1)],
                        lhsT_ap(ct, tap),
                        rhs_ap(n, ct, tap, bk),
                        start=(tap == 0 and ct == 0),
                        stop=(tap == 8 and ct == 3),
                    )

                if order == "pair":
                    # weight-stationary across batch pairs: 4 MMs per
                    # weight tile; 2 PSUM tiles live per group, 2 draining
                    # (use pbufs=2: per-buf = 2 tiles = 4 banks)
                    for g in range(B // 2):
                        psums = [ppool.tile([128, 1024], F32, name="psp")
                                 for _ in range(2)]
                        if skip_mm:
                            for p in psums:
                                nc.vector.memset(p, 0.0)
                        else:
                            for tap in range(9):
                                for ct in range(NCT):
                                    for i in range(2):
                                        for bk in range(2):
                                            mm(psums[i], 2 * g + i,
                                               ct, tap, bk)
                        for i in range(2):
                            store(t, 2 * g + i, psums[i])
                elif order == "quad":
                    # weight-stationary across all 4 batches: 8 MMs per
                    # weight tile; all 4 PSUM tiles live (needs pbufs=4)
                    psums = [ppool.tile([128, 1024], F32, name=f"ps{i}")
                             for i in range(B)]
                    if skip_mm:
                        for p in psums:
                            nc.vector.memset(p, 0.0)
                    else:
                        for tap in range(9):
                            for ct in range(NCT):
                                for n in range(B):
                                    for bk in range(2):
                                        mm(psums[n], n, ct, tap, bk)
                    for n in range(B):
                        store(t, n, psums[n])
                else:
                    for n in range(B):
                        psum = ppool.tile([128, 1024], F32)
                        if skip_mm:
                            nc.vector.memset(psum, 0.0)
                        elif order == "bki":
                            # bk innermost: each weight tile loaded once
                            for tap in range(9):
                                for ct in range(NCT):
                                    for bk in range(2):
                                        mm(psum, n, ct, tap, bk)
                        else:
                            for bk in range(2):
                                for tap in range(9):
                                    for ct in range(NCT):
                                        mm(psum, n, ct, tap, bk)
                        store(t, n, psum)

        if rep == 1:
            body()
        else:
            with tc.For_i(0, rep):
                body()

    nc.compile()
    return nc


def _host_prep(x, W, b):
    xpad = np.pad(np.asarray(x, dtype=np.float32),
                  ((0, 0), (0, 0), (1, 1), (1, 1)), mode="edge")
    # [B, NCT, 128, 34, 34] -> [128, B, NCT, 34, 34]
    xr = np.ascontiguousarray(
        xpad.reshape(B, NCT, 128, HP, WP).transpose(2, 0, 1, 3, 4)
    ).astype(NPBF16)
    # slab[ci, n, ct, bk, a, 4*hp + r] = xpad[n, ..., hp, 4*bk + 8*r + a]
    xs = np.zeros((128, B, NCT, 2, 8, HP, 4), NPBF16)
    for bk in range(2):
        for a in range(8):
            for r in range(4):
                col = 8 * r + a
                wp = 4 * bk + col
                if col < 30 and wp < WP:
                    xs[:, :, :, bk, a, :, r] = xr[:, :, :, :, wp]
    xs = np.ascontiguousarray(xs.reshape(128, XFREE))

    W = np.asarray(W, dtype=np.float32)
    b = np.asarray(b, dtype=np.float32)

    in_maps = []
    for i in range(N_CORES):
        Ws = W[i * COUT_CORE:(i + 1) * COUT_CORE]          # [1216,512,3,3]
        Wp = np.zeros((NTILES * 128, CIN, 3, 3), np.float32)
        Wp[:COUT_CORE] = Ws
        gather = (np.arange(NTILES)[:, None] * 128 +
                  COUT_IN_TILE[None, :])                   # [10,128]
        Wg = Wp[gather]                                    # [10,128(m),512,3,3]
        Wg = Wg.reshape(NTILES, 128, NCT, 128, 9)          # [t,m,ct,p,tap]
        w_dev = np.ascontiguousarray(
            Wg.transpose(0, 3, 2, 4, 1)).astype(NPBF16)    # [t,p,ct,tap,m]
        w_dev = w_dev.reshape(NTILES, 128, NCT * 9 * 128)

        bp = np.zeros((NTILES * 128,), np.float32)
        bp[:COUT_CORE] = b[i * COUT_CORE:(i + 1) * COUT_CORE]
        bias_dev = np.ascontiguousarray(bp[gather].T)      # [128,10]

        in_maps.append({"xs": xs, "w": w_dev, "bias": bias_dev})
    return in_maps


def _run(in_maps, trace=False):
    global _nc_cache
    if _nc_cache is None:
        _nc_cache = _build_nc()
    return run_bass_kernel_spmd(_nc_cache, in_maps,
                                core_ids=list(range(N_CORES)), trace=trace)


def kernel(x, W, b):
    in_maps = _host_prep(x, W, b)
    res = _run(in_maps)
    outs = [res.results[i]["out"] for i in range(N_CORES)]  # [4,19,256,256]
    full = np.concatenate(outs, axis=1)                     # [4,152,256,256]
    return full
